# revision 1
# baseline (speedup 1.0000x reference)
"""Trainium2 Bass kernel for nn_BatchConv1d (dynamic grouped conv attention).

Reference computation (per batch b):
    kernel = (q @ W_kernel + b_kernel).reshape(Lq, C, KW)      # dynamic conv kernels
    bias   = (q @ W_bias + b_bias)[:, 0]
    kpad   = zero-pad k along L by PAD=1
    a[i,j] = sum_{c,w} kernel[i,c,w] * kpad[j+w,c] + bias[i] + bias_b

Strategy: data-parallel over B=8 (one batch per NeuronCore). Per core:
  Stage 1 (PE): kernelT_ext[cw, i] = sum_d Wp_ext[d, cw] * qT[d, i]
     with host-permuted Wp_ext so cw = w*C + c, plus a 13th M-tile holding
     W_bias (row 0) -> bias row. qT built on-chip via PE transposes.
  Stage 2 (PE): out[i, j] = sum_{ct,w} kernelT[w*4+ct][:, i] . kT_pad[ct][:, j+w]
     kT_pad is the on-chip transpose of k with one zero column on each side,
     so the 3 window shifts are just free-dim offsets. b_kernel is folded into
     kernelT during the PSUM->SBUF copy; the per-query bias (row form from
     stage 1, column form via 8 tiny K=1 matmuls) is added during the output
     PSUM->SBUF copy.
All matmuls run as float32r (TF32-like, ~1.17 cyc/row on the PE vs 5.15 for
fp32); inputs are rounded to f32r by the PSUM-copy casts (q, k) or by a
SWDGE DMA cast (W). DMA and PE work are interleaved so stage 1 starts while
later inputs are still in flight.
"""

import numpy as np
from contextlib import ExitStack

import concourse.bass as bass
import concourse.mybir as mybir
import concourse.tile as tile
from concourse import bacc
from concourse.bass_utils import run_bass_kernel_spmd
from concourse.masks import make_identity

F32 = mybir.dt.float32
F32R = mybir.dt.float32r

B, Lq, Lk, D, C, KW = 8, 1024, 1024, 512, 512, 3
CW = C * KW            # 1536
CW_EXT = CW + 128      # 1664 (13th tile: W_bias col + 127 zero cols)
NT_I = Lq // 128       # 8 i-tiles
NT_D = D // 128        # 4 d-tiles
NT_CW = CW_EXT // 128  # 13 cw-tiles (12 kernel + 1 bias)
NJ = 2                 # j chunks of 512

_CACHE = {}


def _build(repeats=1):
    nc = bacc.Bacc(target_bir_lowering=False, debug=False)

    q_in = nc.dram_tensor("q_in", [Lq, D], F32, kind="ExternalInput").ap()
    k_in = nc.dram_tensor("k_in", [Lk, C], F32, kind="ExternalInput").ap()
    wp_in = nc.dram_tensor("wp_in", [D, CW_EXT], F32, kind="ExternalInput").ap()
    bkp_in = nc.dram_tensor("bkp_in", [128, NT_CW - 1], F32, kind="ExternalInput").ap()
    bconst_in = nc.dram_tensor("bconst_in", [1, 1], F32, kind="ExternalInput").ap()
    out = nc.dram_tensor("out", [Lq, Lk], F32, kind="ExternalOutput").ap()

    with tile.TileContext(nc) as tc:
        for rep in range(repeats):
            _emit_body(nc, tc, rep, q_in, k_in, wp_in, bkp_in, bconst_in, out)

    nc.compile()
    return nc


def _emit_body(nc, tc, rep, q_in, k_in, wp_in, bkp_in, bconst_in, out):
    R = f"r{rep}_"
    with ExitStack() as ctx:
        persist = ctx.enter_context(tc.tile_pool(name=R + "persist", bufs=1))
        out_pool = ctx.enter_context(tc.tile_pool(name=R + "outp", bufs=4))

        # identity first: Pool engine work gates the first PE transpose
        ident = persist.tile([128, 128], F32, tag="ident")
        make_identity(nc, ident[:])

        # ---- input DMAs, interleaved to match PE consumption order ----------
        # tiny constants first, then q/wp/k chunks woven so stage-1 can start
        # while later inputs are still in flight
        bkp_col = persist.tile([128, NT_CW - 1], F32, tag="bkp")
        bconst_col = persist.tile([128, 1], F32, tag="bconst")
        nc.vector.memset(bconst_col[:], 0.0)
        zero_col = persist.tile([128, 1], F32, tag="zero_col")
        nc.vector.memset(zero_col[:], 0.0)
        one_t = persist.tile([1, 1], F32, tag="one_t")
        nc.vector.memset(one_t[:], 1.0)

        q_tiled = q_in.rearrange("(t p) d -> t p d", p=128)
        k_tiled = k_in.rearrange("(t p) c -> t p c", p=128)
        wp_tiled = wp_in.rearrange("(t p) m -> t p m", p=128)
        q_sb = [persist.tile([128, D], F32, tag=f"q{t}", name=R + f"q{t}")
                for t in range(NT_I)]
        k_sb = [persist.tile([128, C], F32, tag=f"k{t}", name=R + f"k{t}")
                for t in range(NT_I)]
        wp_sb = [persist.tile([128, CW_EXT], F32R, tag=f"wp{t}", name=R + f"wp{t}")
                 for t in range(NT_D)]
        # wp column sections (by mt group): [0:512], [512:1024], [1024:1536],
        # [1536:1664]; SWDGE cast fp32->f32r
        wp_secs = [(0, 512), (512, 1024), (1024, 1536), (1536, CW_EXT)]

        def dma_wp_sec(s):
            lo, hi = wp_secs[s]
            for t in range(NT_D):
                nc.gpsimd.dma_start(wp_sb[t][:, lo:hi], wp_tiled[t][:, lo:hi])

        for dt in range(NT_D):
            nc.sync.dma_start(q_sb[0][:, dt * 128:(dt + 1) * 128],
                              q_tiled[0][:, dt * 128:(dt + 1) * 128])
        for t in range(1, 4):
            nc.sync.dma_start(q_sb[t][:], q_tiled[t])
        nc.sync.dma_start(bkp_col[:], bkp_in[:])
        nc.sync.dma_start(bconst_col[0:1, :], bconst_in[:])
        dma_wp_sec(0)
        for t in range(4):
            nc.sync.dma_start(k_sb[t][:], k_tiled[t])
        dma_wp_sec(3)
        for t in range(4, NT_I):
            nc.sync.dma_start(q_sb[t][:], q_tiled[t])
        dma_wp_sec(1)
        for t in range(4, NT_I):
            nc.sync.dma_start(k_sb[t][:], k_tiled[t])
        dma_wp_sec(2)

        # ---- persistent SBUF targets ---------------------------------------
        # qT_all[p, dt*Lq + i] = q[i, dt*128+p]; kT_all[p, ct*(Lk+2) + 1 + j] = k[j, ct*128+p]
        qT_all = persist.tile([128, NT_D * Lq], F32R, tag="qT_all")
        kT_all = persist.tile([128, NT_D * (Lk + 2)], F32R, tag="kT_all")
        qT = [qT_all[:, d * Lq:(d + 1) * Lq] for d in range(NT_D)]
        kT_pad = [kT_all[:, c * (Lk + 2):(c + 1) * (Lk + 2)] for c in range(NT_D)]
        kernelT = [
            persist.tile([128, Lq], F32R, tag=f"kern{t}", name=R + f"kern{t}")
            for t in range(NT_CW - 1)
        ]
        bias_row = persist.tile([1, Lq], F32, tag="bias_row")
        for ct in range(NT_D):
            nc.vector.tensor_copy(kT_pad[ct][:, 0:1], zero_col[:])
            nc.vector.tensor_copy(kT_pad[ct][:, Lk + 1:Lk + 2], zero_col[:])

        tp_ctx = tc.tile_pool(name=R + "tpsum", bufs=2, space="PSUM")
        s1_ctx = tc.tile_pool(name=R + "s1psum", bufs=3, space="PSUM")
        tpsum = tp_ctx.__enter__()
        s1psum = s1_ctx.__enter__()

        qT_view = qT_all[:].rearrange("p (d i) -> p d i", d=NT_D)

        def emit_qT(its):
            for it in its:
                pt = tpsum.tile([128, 512], F32, tag="tp", name=R + "tp")
                for dt in range(NT_D):
                    nc.tensor.transpose(
                        pt[:, dt * 128:(dt + 1) * 128],
                        q_sb[it][:, dt * 128:(dt + 1) * 128], ident[:]
                    )
                nc.vector.tensor_copy(
                    qT_view[:, :, it * 128:(it + 1) * 128],
                    pt[:].rearrange("p (d i) -> p d i", d=NT_D),
                )

        kT_view = kT_all[:].rearrange("p (c x) -> p c x", c=NT_D)

        def emit_kT(jts):
            for jt in jts:
                pt = tpsum.tile([128, 512], F32, tag="tp", name=R + "tp")
                for ct in range(NT_D):
                    nc.tensor.transpose(
                        pt[:, ct * 128:(ct + 1) * 128],
                        k_sb[jt][:, ct * 128:(ct + 1) * 128], ident[:]
                    )
                nc.vector.tensor_copy(
                    kT_view[:, :, 1 + jt * 128:1 + (jt + 1) * 128],
                    pt[:].rearrange("p (c i) -> p c i", c=NT_D),
                )

        def emit_s1(mts, njcs):
            for mt in mts:
                for njc in njcs:
                    ps = s1psum.tile([128, 512], F32, tag="s1", name=R + "s1")
                    for dt in range(NT_D):
                        nc.tensor.matmul(
                            ps[:],
                            wp_sb[dt][:, mt * 128:(mt + 1) * 128],
                            qT[dt][:, njc * 512:(njc + 1) * 512],
                            start=(dt == 0),
                            stop=(dt == NT_D - 1),
                        )
                    if mt < NT_CW - 1:
                        nc.any.tensor_scalar_add(
                            kernelT[mt][:, njc * 512:(njc + 1) * 512],
                            ps[:], bkp_col[:, mt:mt + 1]
                        )
                    else:
                        nc.vector.tensor_scalar_add(
                            bias_row[:, njc * 512:(njc + 1) * 512],
                            ps[0:1, :], bconst_col[0:1, :]
                        )

        # PE program order, woven against DMA arrivals; stage-2 for i<512
        # needs only the njc=0 half of stage 1, so it runs early and the
        # njc=1 half of stage 1 overlaps it
        emit_qT(range(0, 4))
        emit_s1(range(0, 4), [0])
        emit_kT(range(0, 4))
        emit_qT(range(4, NT_I))
        emit_s1([NT_CW - 1], [0, 1])
        emit_s1(range(4, 12), [0])

        # bias row -> column form via 8 tiny K=1 matmuls (seg.T @ [1.])
        bias_ps = s1psum.tile([128, NT_I], F32, tag="bias_ps", bufs=1)
        for t in range(NT_I):
            nc.tensor.matmul(
                bias_ps[:, t:t + 1],
                bias_row[:, t * 128:(t + 1) * 128],
                one_t[:],
                start=True, stop=True,
            )
        bias_col = persist.tile([128, NT_I], F32, tag="bias_col")
        nc.vector.tensor_copy(bias_col[:], bias_ps[:])

        # ---- stage 2: out[i, j] = conv matmuls; bias added during copy ------
        with tc.tile_pool(name=R + "s2psum", bufs=2, space="PSUM") as s2psum:
            def emit_s2(its):
                for it in its:
                    for jc in range(NJ):
                        ps = s2psum.tile([128, 512], F32, tag="s2",
                                         name=R + "s2")
                        idx = 0
                        for w in range(KW):
                            for ct in range(NT_D):
                                nc.tensor.matmul(
                                    ps[:],
                                    kernelT[w * NT_D + ct][:, it * 128:(it + 1) * 128],
                                    kT_pad[ct][:, jc * 512 + w:jc * 512 + w + 512],
                                    start=(idx == 0),
                                    stop=(idx == KW * NT_D - 1),
                                )
                                idx += 1
                        o_sb = out_pool.tile([128, 512], F32, tag="osb",
                                             name=R + "osb")
                        nc.any.tensor_scalar_add(o_sb[:], ps[:],
                                                 bias_col[:, it:it + 1])
                        nc.sync.dma_start(
                            out[it * 128:(it + 1) * 128,
                                jc * 512:(jc + 1) * 512],
                            o_sb[:],
                        )

            emit_kT(range(4, NT_I))
            emit_s2(range(0, 4))
            emit_s1(range(0, 12), [1])
            emit_s2(range(4, NT_I))
        s1_ctx.__exit__(None, None, None)
        tp_ctx.__exit__(None, None, None)


def _get_nc():
    if "nc" not in _CACHE:
        _CACHE["nc"] = _build()
    return _CACHE["nc"]


def _prepare_in_maps(q, k, W_kernel, b_kernel, W_bias, b_bias, bias_b):
    q = np.asarray(q, dtype=np.float32)
    k = np.asarray(k, dtype=np.float32)
    W_kernel = np.asarray(W_kernel, dtype=np.float32)
    b_kernel = np.asarray(b_kernel, dtype=np.float32)
    W_bias = np.asarray(W_bias, dtype=np.float32)
    b_bias = np.asarray(b_bias, dtype=np.float32)
    bias_b = np.asarray(bias_b, dtype=np.float32)

    # host-side permutation: Wp[:, w*C + c] = W_kernel[:, c*KW + w]
    Wp = W_kernel.reshape(D, C, KW).transpose(0, 2, 1).reshape(D, CW)
    Wp_ext = np.concatenate(
        [Wp, W_bias.reshape(D, 1), np.zeros((D, 127), np.float32)], axis=1
    )
    Wp_ext = np.ascontiguousarray(Wp_ext, dtype=np.float32)
    bkp = b_kernel.reshape(C, KW).T.reshape(CW)
    bkp_col = np.ascontiguousarray(bkp.reshape(NT_CW - 1, 128).T, dtype=np.float32)
    bconst = np.array([[b_bias.reshape(-1)[0] + bias_b.reshape(-1)[0]]], np.float32)

    return [
        {
            "q_in": np.ascontiguousarray(q[b]),
            "k_in": np.ascontiguousarray(k[b]),
            "wp_in": Wp_ext,
            "bkp_in": bkp_col,
            "bconst_in": bconst,
        }
        for b in range(B)
    ]


def kernel(q, k, W_kernel, b_kernel, W_bias, b_bias, bias_b):
    in_maps = _prepare_in_maps(q, k, W_kernel, b_kernel, W_bias, b_bias, bias_b)
    res = run_bass_kernel_spmd(_get_nc(), in_maps, core_ids=list(range(B)))
    return np.stack([res.results[b]["out"] for b in range(B)], axis=0)


def kernel_profiled(q, k, W_kernel, b_kernel, W_bias, b_bias, bias_b, **kw):
    """Like kernel() but with NTFF tracing; returns (output, BassKernelResults)."""
    in_maps = _prepare_in_maps(q, k, W_kernel, b_kernel, W_bias, b_bias, bias_b)
    res = run_bass_kernel_spmd(
        _get_nc(), in_maps, core_ids=list(range(B)), trace=True, **kw
    )
    out = np.stack([res.results[b]["out"] for b in range(B)], axis=0)
    return out, res



# revision 6
# speedup vs baseline: 3.3390x; 3.3390x over previous
"""Trainium2 Bass kernel for nn_BatchConv1d (dynamic grouped conv attention).

Reference computation (per batch b):
    kernel = (q @ W_kernel + b_kernel).reshape(Lq, C, KW)      # dynamic conv kernels
    bias   = (q @ W_bias + b_bias)[:, 0]
    kpad   = zero-pad k along L by PAD=1
    a[i,j] = sum_{c,w} kernel[i,c,w] * kpad[j+w,c] + bias[i] + bias_b

Strategy: data-parallel over B=8 (one batch per NeuronCore). Per core:
  Inputs ship from host as bf16 (q, k, host-permuted W) so the XBAR DMA
  transpose delivers qT[d, i] and kT[c, j] straight into SBUF -- the PE does
  zero transposes and runs only bf16 matmuls at 1 cyc/row:
  Stage 1 (PE): kernelT[cw, i] = sum_d Wp[d, cw] * qT[d, i], with Wp host-
     permuted so cw = w*C + c and an extra column 1536 holding W_bias whose
     output row is the per-query bias. b_kernel is folded in during the
     PSUM->SBUF copy (DVE, cast to bf16); the bias row gets b_bias + bias_b.
  Stage 2 (PE): out[i, j] = sum_{ct,w} kernelT[w*4+ct][:, i] . kT_pad[ct][:, j+w]
     kT_pad (k shipped host-padded: zero row each side of the XBAR transpose)
     makes the 3 window shifts free-dim offsets; the per-query bias (column form via 8 tiny K=1 matmuls) is added
     during the output PSUM->SBUF copy (Activation engine), whose queue also
     carries the output DMA so the write dispatch needs no extra sync.
  All input tiles double-buffer across repeats (ping/pong by rep parity), so
  the next rep's DMAs dispatch and transfer entirely under the current rep's
  compute and the PE rolls across rep boundaries without idling.  Input DMAs
  ride the SP HWDGE queue; output DMAs ride the Activation HWDGE queue.
"""

import numpy as np
import ml_dtypes
from contextlib import ExitStack

import concourse.bass as bass
import concourse.mybir as mybir
import concourse.tile as tile
from concourse import bacc
from concourse.bass_utils import run_bass_kernel_spmd

F32 = mybir.dt.float32
BF16 = mybir.dt.bfloat16
ACT_IDENT = mybir.ActivationFunctionType.Identity

B, Lq, Lk, D, C, KW = 8, 1024, 1024, 512, 512, 3
CW = C * KW            # 1536
CWE = CW + 1           # 1537 (last col = W_bias)
NT_I = Lq // 128       # 8 i-tiles
NT_D = D // 128        # 4 d-tiles
NT_CW = CW // 128      # 12 kernel col tiles
NJ = 2                 # j chunks of 512
LKP = 1040             # kT row length: 1026 padded rows rounded to 16 for XBAR

_CACHE = {}


def _build(repeats=1):
    nc = bacc.Bacc(target_bir_lowering=False, debug=False)

    q_in = nc.dram_tensor("q_in", [Lq, D], BF16, kind="ExternalInput").ap()
    k_in = nc.dram_tensor("k_in", [LKP, C], BF16, kind="ExternalInput").ap()
    wp_in = nc.dram_tensor("wp_in", [D, CWE], BF16, kind="ExternalInput").ap()
    # [128, 13]: cols 0..11 = b_kernel tiles, col 12 row 0 = b_bias + bias_b
    bkp_in = nc.dram_tensor("bkp_in", [128, NT_CW + 1], F32, kind="ExternalInput").ap()
    out = nc.dram_tensor("out", [Lq, Lk], F32, kind="ExternalOutput").ap()

    with tile.TileContext(nc) as tc, ExitStack() as ctx:
        persist = ctx.enter_context(tc.tile_pool(name="persist", bufs=1))
        out_pool = ctx.enter_context(tc.tile_pool(name="outp", bufs=3))
        s1psum = ctx.enter_context(tc.tile_pool(name="s1psum", bufs=3, space="PSUM"))
        s2psum = ctx.enter_context(tc.tile_pool(name="s2psum", bufs=3, space="PSUM"))

        st = {}
        st["qT"] = [persist.tile([128, NT_D * Lq], BF16, tag=f"qT{p}", name=f"qT{p}")
                    for p in range(2)]
        st["kT"] = [persist.tile([128, NT_D * LKP], BF16, tag=f"kT{p}", name=f"kT{p}")
                    for p in range(2)]
        st["wp"] = [[persist.tile([128, CWE], BF16, tag=f"wp{p}_{t}",
                                  name=f"wp{p}_{t}") for t in range(NT_D)]
                    for p in range(2)]
        st["bkp"] = [persist.tile([128, NT_CW + 1], F32, tag=f"bkp{p}",
                                  name=f"bkp{p}") for p in range(2)]
        st["kernelT"] = [persist.tile([128, Lq], BF16, tag=f"kern{t}", name=f"kern{t}")
                         for t in range(NT_CW)]
        st["bias_row"] = persist.tile([1, Lq], F32, tag="bias_row", name="bias_row")
        st["bias_col"] = persist.tile([128, NT_I], F32, tag="bias_col", name="bias_col")
        st["one_t"] = persist.tile([1, 1], F32, tag="one_t", name="one_t")
        nc.vector.memset(st["one_t"][:], 1.0)

        pools = (out_pool, s1psum, s2psum)
        for rep in range(repeats):
            _emit_rep(nc, rep, st, pools, q_in, k_in, wp_in, bkp_in, out)

    nc.compile()
    return nc


def _emit_rep(nc, rep, st, pools, q_in, k_in, wp_in, bkp_in, out):
    out_pool, s1psum, s2psum = pools
    p = rep % 2
    qT_all, kT_all = st["qT"][p], st["kT"][p]
    wp_sb, bkp_col = st["wp"][p], st["bkp"][p]
    kernelT = st["kernelT"]
    bias_row, bias_col, one_t = st["bias_row"], st["bias_col"], st["one_t"]
    bconst = bkp_col[0:1, NT_CW:NT_CW + 1]

    qT = [qT_all[:, d * Lq:(d + 1) * Lq] for d in range(NT_D)]
    kT_pad = [kT_all[:, c * LKP:(c + 1) * LKP] for c in range(NT_D)]

    # ---- input DMAs (SP queue), in rep-0 consumption order ----------------
    nc.sync.dma_start(bkp_col[:], bkp_in[:])
    for t in range(NT_D):
        nc.sync.dma_start(wp_sb[t][:, 0:512], wp_in[t * 128:(t + 1) * 128, 0:512])
    for dt in range(NT_D):
        nc.sync.dma_start(qT_all[:, dt * Lq:(dt + 1) * Lq],
                          q_in[:, dt * 128:(dt + 1) * 128], transpose=True)
    for t in range(NT_D):
        nc.sync.dma_start(wp_sb[t][:, 512:CWE], wp_in[t * 128:(t + 1) * 128, 512:CWE])
    for ct in range(NT_D):
        nc.sync.dma_start(kT_all[:, ct * LKP:(ct + 1) * LKP],
                          k_in[:, ct * 128:(ct + 1) * 128], transpose=True)

    # ---- stage 1: kernelT[cw, i] (+ bias row via W_bias column) -----------
    def emit_s1(mts, njcs):
        for mt in mts:
            for njc in njcs:
                ps = s1psum.tile([128, 512], F32, tag="s1", name=f"r{rep}s1")
                if mt < NT_CW:
                    for dt in range(NT_D):
                        nc.tensor.matmul(
                            ps[:],
                            wp_sb[dt][:, mt * 128:(mt + 1) * 128],
                            qT[dt][:, njc * 512:(njc + 1) * 512],
                            start=(dt == 0),
                            stop=(dt == NT_D - 1),
                        )
                    nc.vector.tensor_scalar_add(
                        kernelT[mt][:, njc * 512:(njc + 1) * 512],
                        ps[:], bkp_col[:, mt:mt + 1]
                    )
                else:
                    for dt in range(NT_D):
                        nc.tensor.matmul(
                            ps[0:1, :],
                            wp_sb[dt][:, CW:CWE],
                            qT[dt][:, njc * 512:(njc + 1) * 512],
                            start=(dt == 0),
                            stop=(dt == NT_D - 1),
                        )
                    nc.vector.tensor_scalar_add(
                        bias_row[:, njc * 512:(njc + 1) * 512],
                        ps[0:1, :], bconst
                    )

    emit_s1(range(0, NT_CW + 1), [0])

    # bias row half -> column form via 4 tiny K=1 matmuls
    bias_ps = s1psum.tile([128, NT_I], F32, tag="bias_ps", bufs=1)

    def emit_bias_cols(ts):
        for t in ts:
            nc.tensor.matmul(
                bias_ps[:, t:t + 1],
                bias_row[:, t * 128:(t + 1) * 128],
                one_t[:],
                start=True, stop=True,
            )
        nc.vector.tensor_copy(bias_col[:, ts[0]:ts[-1] + 1],
                              bias_ps[:, ts[0]:ts[-1] + 1])

    emit_bias_cols(range(0, 4))

    # ---- stage 2: out[i, j] conv matmuls; bias added in Activation copy ---
    def emit_s2(its):
        for it in its:
            o_sb = out_pool.tile([128, Lk], F32, tag="osb", name=f"r{rep}osb")
            for jc in range(NJ):
                ps = s2psum.tile([128, 512], F32, tag="s2", name=f"r{rep}s2")
                idx = 0
                for w in range(KW):
                    for ct in range(NT_D):
                        nc.tensor.matmul(
                            ps[:],
                            kernelT[w * NT_D + ct][:, it * 128:(it + 1) * 128],
                            kT_pad[ct][:, jc * 512 + w:jc * 512 + w + 512],
                            start=(idx == 0),
                            stop=(idx == KW * NT_D - 1),
                        )
                        idx += 1
                nc.scalar.activation(o_sb[:, jc * 512:(jc + 1) * 512], ps[:],
                                     ACT_IDENT, bias=bias_col[:, it:it + 1])
            nc.scalar.dma_start(out[it * 128:(it + 1) * 128, :], o_sb[:])

    emit_s2(range(0, 4))
    emit_s1(range(0, NT_CW + 1), [1])
    emit_bias_cols(range(4, NT_I))
    emit_s2(range(4, NT_I))


def _get_nc():
    if "nc" not in _CACHE:
        _CACHE["nc"] = _build()
    return _CACHE["nc"]


def _prepare_in_maps(q, k, W_kernel, b_kernel, W_bias, b_bias, bias_b):
    q = np.asarray(q, dtype=np.float32)
    k = np.asarray(k, dtype=np.float32)
    W_kernel = np.asarray(W_kernel, dtype=np.float32)
    b_kernel = np.asarray(b_kernel, dtype=np.float32)
    W_bias = np.asarray(W_bias, dtype=np.float32)
    b_bias = np.asarray(b_bias, dtype=np.float32)
    bias_b = np.asarray(bias_b, dtype=np.float32)

    # host-side permutation: Wp[:, w*C + c] = W_kernel[:, c*KW + w]; col 1536 = W_bias
    Wp = W_kernel.reshape(D, C, KW).transpose(0, 2, 1).reshape(D, CW)
    Wp_ext = np.concatenate([Wp, W_bias.reshape(D, 1)], axis=1)
    wp_bf = np.ascontiguousarray(Wp_ext.astype(ml_dtypes.bfloat16))
    bkp = b_kernel.reshape(C, KW).T.reshape(CW)
    bkp_col = np.zeros((128, NT_CW + 1), np.float32)
    bkp_col[:, :NT_CW] = bkp.reshape(NT_CW, 128).T
    bkp_col[0, NT_CW] = b_bias.reshape(-1)[0] + bias_b.reshape(-1)[0]

    q_bf = q.astype(ml_dtypes.bfloat16)
    # k ships pre-padded: row 0 and rows 1025.. are zero (the conv 'same' pad),
    # row count rounded to 1040 (multiple of 16) for the XBAR DMA transpose
    k_bf = np.zeros((B, LKP, C), ml_dtypes.bfloat16)
    k_bf[:, 1:1 + Lk, :] = k.astype(ml_dtypes.bfloat16)

    return [
        {
            "q_in": np.ascontiguousarray(q_bf[b]),
            "k_in": np.ascontiguousarray(k_bf[b]),
            "wp_in": wp_bf,
            "bkp_in": bkp_col,
        }
        for b in range(B)
    ]


def kernel(q, k, W_kernel, b_kernel, W_bias, b_bias, bias_b):
    in_maps = _prepare_in_maps(q, k, W_kernel, b_kernel, W_bias, b_bias, bias_b)
    res = run_bass_kernel_spmd(_get_nc(), in_maps, core_ids=list(range(B)))
    return np.stack([res.results[b]["out"] for b in range(B)], axis=0)


def kernel_profiled(q, k, W_kernel, b_kernel, W_bias, b_bias, bias_b, **kw):
    """Like kernel() but with NTFF tracing; returns (output, BassKernelResults)."""
    in_maps = _prepare_in_maps(q, k, W_kernel, b_kernel, W_bias, b_bias, bias_b)
    res = run_bass_kernel_spmd(
        _get_nc(), in_maps, core_ids=list(range(B)), trace=True, **kw
    )
    out = np.stack([res.results[b]["out"] for b in range(B)], axis=0)
    return out, res
